# revision 3
# baseline (speedup 1.0000x reference)
"""Trainium2 Bass kernel for nn_Encoder (GRU encoder, reset_after=True).

Shapes (hardcoded): B=64, T=512, V=32000, E=1024, U=1024, 8 NeuronCores.
Sharding: data-parallel over batch (8 rows/core); weights replicated;
time scan local per core.

Per-core plan:
  Phase 1: embedding gather (indirect DMA, 128-token tiles) -> PE transpose
    -> xp = e @ Wx GEMM in float32r (1 cyc/row) -> xp written to DRAM scratch
    in a gate/strip-permuted layout (column permutation folded into Wx host-side).
  Phase 2: 512-step GRU recurrence.
    State kept transposed: hT [128, 8k+b] (bf16, matmul lhsT) and h_strip
    [128, 256] (f32, gate math).  Per step, hp strips computed by 32 bf16
    matmuls whose moving operand is Wh (4 column-tile groups stream
    concurrently); xp_z/xp_r folded into PSUM via an 8x8-identity matmul;
    gates run on full-128-partition strip tiles (sigmoid/tanh straight from
    PSUM); hT rebuilt with one DVE 32x32 stream-transpose + strided copy.

Strip layout [128, 256]: partition 32j+b, free col c' = 128H+32m+i maps to
hidden unit u = 128*(4H+m) + 32j + i.  (See mk_strip_ucols.)
"""
import numpy as np
import ml_dtypes

B, T, V, E, U = 64, 512, 32000, 1024, 1024
NCORES = 8
BL = B // NCORES           # 8 local batch rows
J, NH = 4, 2               # strip groups, column halves
NP_BF16 = ml_dtypes.bfloat16

_C128 = np.arange(128)


def _strip_ucols(j, H):
    return 128 * (4 * H + _C128 // 32) + 32 * j + _C128 % 32


def _round_tf32(a):
    u = np.ascontiguousarray(a, dtype=np.float32).view(np.uint32)
    r = u + 0x00000FFF + ((u >> 13) & 1)
    r &= np.uint32(0xFFFFE000)
    return r.view(np.float32)


def _perm_zr():
    p = np.empty(2048, np.int64)
    for j in range(J):
        for H in range(NH):
            uc = _strip_ucols(j, H)
            for g in range(2):
                p[(j * 2 + H) * 256 + g * 128:(j * 2 + H) * 256 + g * 128 + 128] = g * 1024 + uc
    return p


def _perm_h():
    p = np.empty(1024, np.int64)
    for j in range(J):
        for H in range(NH):
            p[j * 256 + H * 128:j * 256 + H * 128 + 128] = 2 * 1024 + _strip_ucols(j, H)
    return p


def _make_whs(Wh):
    """Wh [U, 3U] -> [8, 128, 3072] bf16; col j*768 + H*384 + g*128 + c."""
    whs = np.empty((8, 128, 3072), np.float32)
    for j in range(J):
        for H in range(NH):
            uc = _strip_ucols(j, H)
            for g in range(3):
                o = j * 768 + H * 384 + g * 128
                whs[:, :, o:o + 128] = Wh[:, g * 1024 + uc].reshape(8, 128, 128)
    return whs.astype(NP_BF16)


def _ht_from_bu(h):
    """h [BL, U] -> hT [128, 64]"""
    return np.ascontiguousarray(h.T.reshape(8, 128, BL).transpose(1, 0, 2).reshape(128, 8 * BL))


def _strip_from_bu(a):
    out = np.zeros((128, 256), np.float32)
    for j in range(J):
        for H in range(NH):
            out[32 * j:32 * j + BL, 128 * H:128 * H + 128] = a[:, _strip_ucols(j, H)]
    return out


_PROG_CACHE = {}


def _build_program(with_bias):
    import concourse.bacc as bacc
    import concourse.mybir as mybir
    import concourse.tile as tile
    import concourse.bass as bass
    from concourse.bass import ds

    f32, f32r, bf16 = mybir.dt.float32, mybir.dt.float32r, mybir.dt.bfloat16
    u32, i32 = mybir.dt.uint32, mybir.dt.int32
    AF = mybir.ActivationFunctionType

    nc = bacc.Bacc("TRN2", target_bir_lowering=False, debug=False, num_devices=NCORES)

    # ---- I/O ----
    x_d = nc.dram_tensor("x", [BL, T], i32, kind="ExternalInput").ap()
    emb_d = nc.dram_tensor("emb", [V, E], f32, kind="ExternalInput").ap()
    wxzr_d = nc.dram_tensor("wx_zr", [E, 2048], f32r, kind="ExternalInput").ap()
    wxh_d = nc.dram_tensor("wx_h", [E, 1024], f32r, kind="ExternalInput").ap()
    whs_d = nc.dram_tensor("whs", [8, 128, 3072], bf16, kind="ExternalInput").ap()
    i8_d = nc.dram_tensor("i8", [8, 8], bf16, kind="ExternalInput").ap()
    ident_d = nc.dram_tensor("ident", [128, 128], f32, kind="ExternalInput").ap()
    ht0_d = nc.dram_tensor("ht0", [128, 8 * BL], bf16, kind="ExternalInput").ap()
    hs0_d = nc.dram_tensor("hs0", [128, 256], f32, kind="ExternalInput").ap()
    mask_d = nc.dram_tensor("mask", [128, T], u32, kind="ExternalInput").ap()
    if with_bias:
        ones_d = nc.dram_tensor("ones_row", [1, 128], f32r, kind="ExternalInput").ap()
        bzr_d = nc.dram_tensor("bias_zr", [1, 2048], f32r, kind="ExternalInput").ap()
        bh_d = nc.dram_tensor("bias_h", [1, 1024], f32r, kind="ExternalInput").ap()
        ones1_d = nc.dram_tensor("ones1", [1, 8], bf16, kind="ExternalInput").ap()
        brh_d = nc.dram_tensor("b_rh", [1, 1024], bf16, kind="ExternalInput").ap()

    out_d = nc.dram_tensor("out", [BL, T, U], f32, kind="ExternalOutput").ap()

    # DRAM scratch for xp
    xpzr_d = nc.dram_tensor("xpzr_scr", [BL, T, 2048], bf16).ap()
    xph_d = nc.dram_tensor("xph_scr", [BL, T, 1024], f32).ap()

    NT = T // 128  # 4 token tiles per batch row

    with tile.TileContext(nc) as tc:
        from contextlib import ExitStack
        with ExitStack() as stk:
            cpool = stk.enter_context(tc.tile_pool(name="const", bufs=1))

            # ---- resident constants ----
            whs = cpool.tile([128, 8 * 3072], bf16, tag="whs")
            for k in range(8):
                nc.sync.dma_start(whs[:, k * 3072:(k + 1) * 3072], whs_d[k])
            i8 = cpool.tile([8, 8], bf16, tag="i8")
            nc.sync.dma_start(i8[:], i8_d)
            mask_sb = cpool.tile([128, T], u32, tag="mask")
            nc.sync.dma_start(mask_sb[:], mask_d)
            if with_bias:
                ones1 = cpool.tile([1, 8], bf16, tag="ones1")
                nc.sync.dma_start(ones1[:], ones1_d)
                brh = cpool.tile([1, 1024], bf16, tag="brh")
                nc.sync.dma_start(brh[:], brh_d)

            # ================= Phase 1 =================
            with ExitStack() as p1:
                p1c = p1.enter_context(tc.tile_pool(name="p1const", bufs=1))
                p1w = p1.enter_context(tc.tile_pool(name="p1work", bufs=3))
                p1o = p1.enter_context(tc.tile_pool(name="p1out", bufs=4))
                p1ps = p1.enter_context(tc.tile_pool(name="p1ps", bufs=3, space="PSUM"))
                p1pt = p1.enter_context(tc.tile_pool(name="p1pt", bufs=2, space="PSUM"))

                wxzr = p1c.tile([128, 8 * 2048], f32r, tag="wxzr")
                wxh = p1c.tile([128, 8 * 1024], f32r, tag="wxh")
                wxzr_r = wxzr_d.rearrange("(k p) n -> k p n", p=128)
                wxh_r = wxh_d.rearrange("(k p) n -> k p n", p=128)
                for k in range(8):
                    nc.sync.dma_start(wxzr[:, k * 2048:(k + 1) * 2048], wxzr_r[k])
                    nc.sync.dma_start(wxh[:, k * 1024:(k + 1) * 1024], wxh_r[k])
                ident = p1c.tile([128, 128], f32, tag="ident")
                nc.sync.dma_start(ident[:], ident_d)
                if with_bias:
                    ones_row = p1c.tile([1, 128], f32r, tag="onesr")
                    nc.sync.dma_start(ones_row[:], ones_d)
                    bzr = p1c.tile([1, 2048], f32r, tag="bzr")
                    nc.sync.dma_start(bzr[:], bzr_d)
                    bh = p1c.tile([1, 1024], f32r, tag="bh")
                    nc.sync.dma_start(bh[:], bh_d)

                for tt in range(NT):
                    for b in range(BL):
                        idx = p1w.tile([128, 1], i32, tag="idx")
                        nc.sync.dma_start(idx[:, 0:1], x_d[b:b + 1, tt * 128:(tt + 1) * 128])
                        e_t = p1w.tile([128, E], f32, tag="et")
                        nc.gpsimd.indirect_dma_start(
                            out=e_t[:], out_offset=None, in_=emb_d[:, :],
                            in_offset=bass.IndirectOffsetOnAxis(ap=idx[:, 0:1], axis=0),
                        )
                        eT = p1w.tile([128, E], f32r, tag="eT")
                        for k in range(8):
                            tp = p1pt.tile([128, 128], f32, tag="tp")
                            nc.tensor.transpose(tp[:], e_t[:, k * 128:(k + 1) * 128], ident[:])
                            nc.vector.tensor_copy(eT[:, k * 128:(k + 1) * 128], tp[:])
                        # zr GEMM: 4 banks of 512
                        for n in range(4):
                            acc = p1ps.tile([128, 512], f32, tag="acc")
                            for k in range(8):
                                nc.tensor.matmul(acc[:], lhsT=eT[:, k * 128:(k + 1) * 128],
                                                 rhs=wxzr[:, k * 2048 + n * 512:k * 2048 + n * 512 + 512],
                                                 start=(k == 0), stop=(k == 7 and not with_bias))
                            if with_bias:
                                nc.tensor.matmul(acc[:], lhsT=ones_row[:],
                                                 rhs=bzr[:, n * 512:(n + 1) * 512],
                                                 start=False, stop=True)
                            ozr = p1o.tile([128, 512], bf16, tag="ozr")
                            nc.vector.tensor_copy(ozr[:], acc[:])
                            nc.sync.dma_start(
                                xpzr_d[b, tt * 128:(tt + 1) * 128, n * 512:(n + 1) * 512], ozr[:])
                        # h GEMM: 2 banks of 512
                        for n in range(2):
                            acc = p1ps.tile([128, 512], f32, tag="acc")
                            for k in range(8):
                                nc.tensor.matmul(acc[:], lhsT=eT[:, k * 128:(k + 1) * 128],
                                                 rhs=wxh[:, k * 1024 + n * 512:k * 1024 + n * 512 + 512],
                                                 start=(k == 0), stop=(k == 7 and not with_bias))
                            if with_bias:
                                nc.tensor.matmul(acc[:], lhsT=ones_row[:],
                                                 rhs=bh[:, n * 512:(n + 1) * 512],
                                                 start=False, stop=True)
                            oh = p1o.tile([128, 512], f32, tag="oh")
                            nc.vector.tensor_copy(oh[:], acc[:])
                            nc.sync.dma_start(
                                xph_d[b, tt * 128:(tt + 1) * 128, n * 512:(n + 1) * 512], oh[:])

            # ================= Phase 2: recurrence =================
            spool = stk.enter_context(tc.tile_pool(name="state", bufs=1))
            lpool = stk.enter_context(tc.tile_pool(name="loop", bufs=6))
            wpool = stk.enter_context(tc.tile_pool(name="work", bufs=2))
            pspool = stk.enter_context(tc.tile_pool(name="ps2", bufs=2, space="PSUM"))

            hT_st = [spool.tile([128, 8 * BL], bf16, name=f"hT{i}", tag=f"hT{i}") for i in range(2)]
            hs_st = [spool.tile([128, 256], f32, name=f"hs{i}", tag=f"hs{i}") for i in range(2)]
            nc.sync.dma_start(hT_st[0][:], ht0_d)
            nc.sync.dma_start(hs_st[0][:], hs0_d)

            UNROLL = 16
            slot = [0]

            def body(iv):
                u = slot[0] % 2
                slot[0] += 1
                hT_prev, hT_new = hT_st[u], hT_st[1 - u]
                hs_prev, hs_new = hs_st[u], hs_st[1 - u]

                xpzr = lpool.tile([BL, 2048], bf16, tag="xpzr")
                nc.sync.dma_start(xpzr[:], xpzr_d[0:BL, ds(iv, 1), :])
                xph = lpool.tile([128, 256], f32, tag="xph")
                for j in range(J):
                    nc.sync.dma_start(xph[32 * j:32 * j + BL, :],
                                      xph_d[0:BL, ds(iv, 1), j * 256:(j + 1) * 256])

                psum = pspool.tile([128, 1024], mybir.dt.float32, tag="acc2")
                for H in range(NH):
                    for k in range(8):
                        for j in range(J):
                            nc.tensor.matmul(
                                psum[32 * j:32 * j + BL, 512 * H:512 * H + 384],
                                lhsT=hT_prev[:, 8 * k:8 * k + 8],
                                rhs=whs[:, (k * 4 + j) * 768 + 384 * H:(k * 4 + j) * 768 + 384 * H + 384],
                                start=(k == 0), stop=False,
                                tile_position=(0, 32 * j))
                    for j in range(J):
                        nc.tensor.matmul(
                            psum[32 * j:32 * j + BL, 512 * H:512 * H + 256],
                            lhsT=i8[:],
                            rhs=xpzr[0:8, (j * 2 + H) * 256:(j * 2 + H) * 256 + 256],
                            start=False, stop=(not with_bias),
                            tile_position=(0, 32 * j))
                    if with_bias:
                        for j in range(J):
                            nc.tensor.matmul(
                                psum[32 * j:32 * j + BL, 512 * H + 256:512 * H + 384],
                                lhsT=ones1[:],
                                rhs=brh[0:1, j * 256 + 128 * H:j * 256 + 128 * H + 128],
                                start=False, stop=True,
                                tile_position=(0, 32 * j))

                for H in range(NH):
                    zr_s = wpool.tile([128, 256], mybir.dt.float32, tag="zrs")
                    nc.scalar.activation(zr_s[:], psum[:, 512 * H:512 * H + 256], AF.Sigmoid)
                    tmp = wpool.tile([128, 128], mybir.dt.float32, tag="tmp")
                    nc.vector.tensor_mul(tmp[:], zr_s[:, 128:256], psum[:, 512 * H + 256:512 * H + 384])
                    tmp2 = wpool.tile([128, 128], mybir.dt.float32, tag="tmp2")
                    nc.vector.tensor_add(tmp2[:], tmp[:], xph[:, 128 * H:128 * H + 128])
                    cand = wpool.tile([128, 128], mybir.dt.float32, tag="cand")
                    nc.scalar.activation(cand[:], tmp2[:], AF.Tanh)
                    dd = wpool.tile([128, 128], mybir.dt.float32, tag="dd")
                    nc.vector.tensor_sub(dd[:], hs_prev[:, 128 * H:128 * H + 128], cand[:])
                    ee = wpool.tile([128, 128], mybir.dt.float32, tag="ee")
                    nc.vector.tensor_mul(ee[:], zr_s[:, 0:128], dd[:])
                    nc.vector.tensor_add(hs_new[:, 128 * H:128 * H + 128], cand[:], ee[:])
                    nc.vector.copy_predicated(hs_new[:, 128 * H:128 * H + 128],
                                              mask_sb[:, ds(iv, 1)].to_broadcast([128, 128]),
                                              hs_prev[:, 128 * H:128 * H + 128])
                    ttmp = wpool.tile([128, 128], mybir.dt.float32, tag="ttmp")
                    nc.vector.transpose(ttmp[:], hs_new[:, 128 * H:128 * H + 128])
                    nc.vector.tensor_copy(
                        hT_new[:, 32 * H:32 * H + 32].rearrange("p (m i) -> p m i", m=4),
                        ttmp[:].rearrange("p (m i) -> p m i", m=4)[:, :, 0:8])

                out_r = out_d[0:BL, ds(iv, 1), :].rearrange("b t (W i) -> b t W i", i=32)
                for j in range(J):
                    nc.sync.dma_start(out_r[:, :, j::4, :], hs_new[32 * j:32 * j + BL, :])

            tc.For_i_unrolled(0, T, 1, body, max_unroll=UNROLL)

    nc.compile()
    return nc


def _get_program(with_bias):
    if with_bias not in _PROG_CACHE:
        _PROG_CACHE[with_bias] = _build_program(with_bias)
    return _PROG_CACHE[with_bias]


def kernel(x, hidden, emb, Wx, Wh, b_i, b_r):
    from concourse.bass_utils import run_bass_kernel_spmd

    x = np.asarray(x)
    hidden = np.asarray(hidden, np.float32)
    emb = np.ascontiguousarray(np.asarray(emb, np.float32))
    Wx = np.asarray(Wx, np.float32)
    Wh = np.asarray(Wh, np.float32)
    b_i = np.asarray(b_i, np.float32)
    b_r = np.asarray(b_r, np.float32)

    with_bias = bool(np.any(b_i) or np.any(b_r))
    nc = _get_program(with_bias)

    pzr, ph = _perm_zr(), _perm_h()
    # fold b_r (z,r parts) + b_i into the phase-1 bias rows
    wxzr_np = _round_tf32(Wx[:, pzr])
    wxh_np = _round_tf32(Wx[:, ph])
    whs_np = _make_whs(Wh)
    i8_np = np.eye(8, dtype=NP_BF16)
    ident_np = np.eye(128, dtype=np.float32)

    common = {
        "emb": emb, "wx_zr": wxzr_np, "wx_h": wxh_np, "whs": whs_np,
        "i8": i8_np, "ident": ident_np,
    }
    if with_bias:
        bias_full = b_i + b_r          # for z,r gates sigma(x+bi + h@W + br)
        common["ones_row"] = np.ones((1, 128), np.float32)
        common["bias_zr"] = _round_tf32(bias_full[pzr][None, :])
        common["bias_h"] = _round_tf32(b_i[ph][None, :])   # b_i_h only
        common["ones1"] = np.ones((1, 8), NP_BF16)
        brh_np = np.empty(1024, np.float32)
        for j in range(J):
            for H in range(NH):
                brh_np[j * 256 + 128 * H:j * 256 + 128 * H + 128] = b_r[2048 + _strip_ucols(j, H)]
        common["b_rh"] = brh_np[None, :].astype(NP_BF16)

    in_maps = []
    for c in range(NCORES):
        xs = np.ascontiguousarray(x[c * BL:(c + 1) * BL]).astype(np.int32)
        hs = hidden[c * BL:(c + 1) * BL]
        mask_np = np.zeros((128, T), np.uint32)
        zi = (xs == 0)
        for j in range(J):
            mask_np[32 * j:32 * j + BL, :] = zi
        in_maps.append({
            **common,
            "x": xs,
            "ht0": _ht_from_bu(hs).astype(NP_BF16),
            "hs0": _strip_from_bu(hs),
            "mask": mask_np,
        })

    res = run_bass_kernel_spmd(nc, in_maps, list(range(NCORES)))
    global LAST_RESULTS
    LAST_RESULTS = res
    out = np.empty((B, T, U), np.float32)
    for c in range(NCORES):
        out[c * BL:(c + 1) * BL] = res.results[c]["out"]
    state = np.ascontiguousarray(out[:, -1, :])
    return out, state


# revision 8
# speedup vs baseline: 1.1204x; 1.1204x over previous
"""Trainium2 Bass kernel for nn_Encoder (GRU encoder, reset_after=True).

Shapes (hardcoded): B=64, T=512, V=32000, E=1024, U=1024, 8 NeuronCores.
Sharding: data-parallel over batch (8 rows/core); weights replicated;
time scan local per core.

Per-core plan:
  Phase 1: embedding gather (indirect DMA, 128-token tiles) -> PE transpose
    -> xp = e @ Wx GEMM in float32r (1 cyc/row) -> xp written to DRAM scratch
    in a gate/strip-permuted layout (column permutation folded into Wx host-side).
    The mask_zero semantics are folded in here: +50 is added to xp_z rows of
    zero tokens, so z = sigmoid(...) saturates to exactly 1.0 and
    h_new = z*h + (1-z)*cand == h for masked steps.
  Phase 2: 512-step GRU recurrence.
    State kept transposed: hT [128, 8k+b] (bf16, matmul lhsT) and h_strip
    [128, 256] (f32, gate math).  Per step, hp strips are computed by 64 bf16
    matmuls whose moving operand is Wh; 4 column-tile groups (tile_position)
    stream concurrently; xp_z/xp_r are accumulated into PSUM via an
    8x8-identity matmul; gates run on full-128-partition strip tiles
    (sigmoid/tanh straight from PSUM, 1-z via sigmoid(-pre)); hT is rebuilt
    with a DVE 32x32 stream-transpose + one strided copy per half.

Strip layout [128, 256]: partition 32j+b, free col c' = 128H+32m+i maps to
hidden unit u = 128*(4H+m) + 32j + i.
"""
import numpy as np
import ml_dtypes

B, T, V, E, U = 64, 512, 32000, 1024, 1024
NCORES = 8
BL = B // NCORES           # 8 local batch rows
J, NH = 4, 2               # strip groups, column halves
NP_BF16 = ml_dtypes.bfloat16
MASK_BIG = 50.0

_C128 = np.arange(128)


def _strip_ucols(j, H):
    return 128 * (4 * H + _C128 // 32) + 32 * j + _C128 % 32


def _round_tf32(a):
    u = np.ascontiguousarray(a, dtype=np.float32).view(np.uint32)
    r = u + 0x00000FFF + ((u >> 13) & 1)
    r &= np.uint32(0xFFFFE000)
    return r.view(np.float32)


def _perm_zr():
    p = np.empty(2048, np.int64)
    for j in range(J):
        for H in range(NH):
            uc = _strip_ucols(j, H)
            for g in range(2):
                p[(j * 2 + H) * 256 + g * 128:(j * 2 + H) * 256 + g * 128 + 128] = g * 1024 + uc
    return p


def _perm_h():
    p = np.empty(1024, np.int64)
    for j in range(J):
        for H in range(NH):
            p[j * 256 + H * 128:j * 256 + H * 128 + 128] = 2 * 1024 + _strip_ucols(j, H)
    return p


def _make_whs(Wh):
    """Wh [U, 3U] -> [8, 128, 3072] bf16; col j*768 + H*384 + g*128 + c."""
    whs = np.empty((8, 128, 3072), np.float32)
    for j in range(J):
        for H in range(NH):
            uc = _strip_ucols(j, H)
            for g in range(3):
                o = j * 768 + H * 384 + g * 128
                whs[:, :, o:o + 128] = Wh[:, g * 1024 + uc].reshape(8, 128, 128)
    return whs.astype(NP_BF16)


def _ht_from_bu(h):
    """h [BL, U] -> hT [128, 64]"""
    return np.ascontiguousarray(h.T.reshape(8, 128, BL).transpose(1, 0, 2).reshape(128, 8 * BL))


def _strip_from_bu(a):
    out = np.zeros((128, 256), np.float32)
    for j in range(J):
        for H in range(NH):
            out[32 * j:32 * j + BL, 128 * H:128 * H + 128] = a[:, _strip_ucols(j, H)]
    return out


_PROG_CACHE = {}


def _build_program(with_bias):
    import concourse.bacc as bacc
    import concourse.mybir as mybir
    import concourse.tile as tile
    import concourse.bass as bass
    from concourse.bass import ds
    from contextlib import ExitStack

    f32, f32r, bf16 = mybir.dt.float32, mybir.dt.float32r, mybir.dt.bfloat16
    i32 = mybir.dt.int32
    AF = mybir.ActivationFunctionType
    ALU = mybir.AluOpType

    nc = bacc.Bacc("TRN2", target_bir_lowering=False, debug=False, num_devices=NCORES)

    # ---- I/O ----
    x_d = nc.dram_tensor("x", [BL, T], i32, kind="ExternalInput").ap()
    maskb_d = nc.dram_tensor("maskb", [BL, T], f32, kind="ExternalInput").ap()
    emb_d = nc.dram_tensor("emb", [V, E], f32, kind="ExternalInput").ap()
    wxzr_d = nc.dram_tensor("wx_zr", [E, 2048], f32r, kind="ExternalInput").ap()
    wxh_d = nc.dram_tensor("wx_h", [E, 1024], f32r, kind="ExternalInput").ap()
    whs_d = nc.dram_tensor("whs", [8, 128, 3072], bf16, kind="ExternalInput").ap()
    i8_d = nc.dram_tensor("i8", [8, 8], bf16, kind="ExternalInput").ap()
    ident_d = nc.dram_tensor("ident", [128, 128], f32, kind="ExternalInput").ap()
    ht0_d = nc.dram_tensor("ht0", [128, 8 * BL], bf16, kind="ExternalInput").ap()
    hs0_d = nc.dram_tensor("hs0", [128, 256], f32, kind="ExternalInput").ap()
    if with_bias:
        ones_d = nc.dram_tensor("ones_row", [1, 128], f32r, kind="ExternalInput").ap()
        bzr_d = nc.dram_tensor("bias_zr", [1, 2048], f32r, kind="ExternalInput").ap()
        bh_d = nc.dram_tensor("bias_h", [1, 1024], f32r, kind="ExternalInput").ap()
        ones1_d = nc.dram_tensor("ones1", [1, 8], bf16, kind="ExternalInput").ap()
        brh_d = nc.dram_tensor("b_rh", [1, 1024], bf16, kind="ExternalInput").ap()

    out_d = nc.dram_tensor("out", [BL, T, U], f32, kind="ExternalOutput").ap()

    # DRAM scratch for xp
    xpzr_d = nc.dram_tensor("xpzr_scr", [BL, T, 2048], bf16).ap()
    xph_d = nc.dram_tensor("xph_scr", [BL, T, 1024], f32).ap()

    NT = T // 128  # 4 token tiles per batch row

    with tile.TileContext(nc) as tc:
        with ExitStack() as stk:
            cpool = stk.enter_context(tc.tile_pool(name="const", bufs=1))

            # ---- resident constants ----
            whs = cpool.tile([128, 8 * 3072], bf16, tag="whs")
            for k in range(8):
                nc.sync.dma_start(whs[:, k * 3072:(k + 1) * 3072], whs_d[k])
            i8 = cpool.tile([8, 8], bf16, tag="i8")
            nc.sync.dma_start(i8[:], i8_d)
            if with_bias:
                ones1 = cpool.tile([1, 8], bf16, tag="ones1")
                nc.sync.dma_start(ones1[:], ones1_d)
                brh = cpool.tile([1, 1024], bf16, tag="brh")
                nc.sync.dma_start(brh[:], brh_d)

            # ================= Phase 1 =================
            with ExitStack() as p1:
                p1c = p1.enter_context(tc.tile_pool(name="p1const", bufs=1))
                p1w = p1.enter_context(tc.tile_pool(name="p1work", bufs=3))
                p1o = p1.enter_context(tc.tile_pool(name="p1out", bufs=4))
                p1ps = p1.enter_context(tc.tile_pool(name="p1ps", bufs=3, space="PSUM"))
                p1pt = p1.enter_context(tc.tile_pool(name="p1pt", bufs=2, space="PSUM"))

                wxzr = p1c.tile([128, 8 * 2048], f32r, tag="wxzr")
                wxh = p1c.tile([128, 8 * 1024], f32r, tag="wxh")
                wxzr_r = wxzr_d.rearrange("(k p) n -> k p n", p=128)
                wxh_r = wxh_d.rearrange("(k p) n -> k p n", p=128)
                for k in range(8):
                    nc.sync.dma_start(wxzr[:, k * 2048:(k + 1) * 2048], wxzr_r[k])
                    nc.sync.dma_start(wxh[:, k * 1024:(k + 1) * 1024], wxh_r[k])
                ident = p1c.tile([128, 128], f32, tag="ident")
                nc.sync.dma_start(ident[:], ident_d)
                if with_bias:
                    ones_row = p1c.tile([1, 128], f32r, tag="onesr")
                    nc.sync.dma_start(ones_row[:], ones_d)
                    bzr = p1c.tile([1, 2048], f32r, tag="bzr")
                    nc.sync.dma_start(bzr[:], bzr_d)
                    bh = p1c.tile([1, 1024], f32r, tag="bh")
                    nc.sync.dma_start(bh[:], bh_d)

                for tt in range(NT):
                    for b in range(BL):
                        idx = p1w.tile([128, 1], i32, tag="idx")
                        nc.sync.dma_start(idx[:, 0:1], x_d[b:b + 1, tt * 128:(tt + 1) * 128])
                        mb = p1w.tile([128, 1], f32, tag="mb")
                        nc.sync.dma_start(mb[:, 0:1], maskb_d[b:b + 1, tt * 128:(tt + 1) * 128])
                        e_t = p1w.tile([128, E], f32, tag="et")
                        nc.gpsimd.indirect_dma_start(
                            out=e_t[:], out_offset=None, in_=emb_d[:, :],
                            in_offset=bass.IndirectOffsetOnAxis(ap=idx[:, 0:1], axis=0),
                        )
                        eT = p1w.tile([128, E], f32r, tag="eT")
                        for k in range(8):
                            tp = p1pt.tile([128, 128], f32, tag="tp")
                            nc.tensor.transpose(tp[:], e_t[:, k * 128:(k + 1) * 128], ident[:])
                            nc.vector.tensor_copy(eT[:, k * 128:(k + 1) * 128], tp[:])
                        # zr GEMM: 4 psum banks of 512 (= 2 (j,H) blocks each)
                        for n in range(4):
                            acc = p1ps.tile([128, 512], f32, tag="acc")
                            for k in range(8):
                                nc.tensor.matmul(acc[:], lhsT=eT[:, k * 128:(k + 1) * 128],
                                                 rhs=wxzr[:, k * 2048 + n * 512:k * 2048 + n * 512 + 512],
                                                 start=(k == 0), stop=(k == 7 and not with_bias))
                            if with_bias:
                                nc.tensor.matmul(acc[:], lhsT=ones_row[:],
                                                 rhs=bzr[:, n * 512:(n + 1) * 512],
                                                 start=False, stop=True)
                            ozr = p1o.tile([128, 512], bf16, tag="ozr")
                            # z columns (g=0 slices) get the +MASK_BIG per-token bias
                            acc_r = acc[:].rearrange("p (blk g c) -> p blk g c", blk=2, g=2)
                            ozr_r = ozr[:].rearrange("p (blk g c) -> p blk g c", blk=2, g=2)
                            nc.vector.tensor_scalar(
                                out=ozr_r[:, :, 0, :], in0=acc_r[:, :, 0, :],
                                scalar1=mb[:, 0:1], scalar2=None, op0=ALU.add)
                            nc.vector.tensor_copy(ozr_r[:, :, 1, :], acc_r[:, :, 1, :])
                            nc.sync.dma_start(
                                xpzr_d[b, tt * 128:(tt + 1) * 128, n * 512:(n + 1) * 512], ozr[:])
                        # h GEMM: 2 psum banks of 512
                        for n in range(2):
                            acc = p1ps.tile([128, 512], f32, tag="acc")
                            for k in range(8):
                                nc.tensor.matmul(acc[:], lhsT=eT[:, k * 128:(k + 1) * 128],
                                                 rhs=wxh[:, k * 1024 + n * 512:k * 1024 + n * 512 + 512],
                                                 start=(k == 0), stop=(k == 7 and not with_bias))
                            if with_bias:
                                nc.tensor.matmul(acc[:], lhsT=ones_row[:],
                                                 rhs=bh[:, n * 512:(n + 1) * 512],
                                                 start=False, stop=True)
                            oh = p1o.tile([128, 512], f32, tag="oh")
                            nc.vector.tensor_copy(oh[:], acc[:])
                            nc.sync.dma_start(
                                xph_d[b, tt * 128:(tt + 1) * 128, n * 512:(n + 1) * 512], oh[:])

            # ================= Phase 2: recurrence =================
            spool = stk.enter_context(tc.tile_pool(name="state", bufs=1))
            lpool = stk.enter_context(tc.tile_pool(name="loop", bufs=8))
            wpool = stk.enter_context(tc.tile_pool(name="work", bufs=3))
            pspool = stk.enter_context(tc.tile_pool(name="ps2", bufs=2, space="PSUM"))

            hT_st = [spool.tile([128, 8 * BL], bf16, name=f"hT{i}", tag=f"hT{i}") for i in range(2)]
            hs_st = [spool.tile([128, 256], f32, name=f"hs{i}", tag=f"hs{i}") for i in range(2)]
            nc.sync.dma_start(hT_st[0][:], ht0_d)
            nc.sync.dma_start(hs_st[0][:], hs0_d)

            UNROLL = 64
            slot = [0]

            def body(iv):
                u = slot[0] % 2
                slot[0] += 1
                hT_prev, hT_new = hT_st[u], hT_st[1 - u]
                hs_prev, hs_new = hs_st[u], hs_st[1 - u]

                xpzr = lpool.tile([BL, 2048], bf16, tag="xpzr")
                nc.sync.dma_start(xpzr[:], xpzr_d[0:BL, ds(iv, 1), :])
                # xph spread: one DMA, dst partitions (j, b), src [j][b][c]
                xph = lpool.tile([128, 256], mybir.dt.float32, tag="xph")
                for j in range(J):
                    nc.scalar.dma_start(xph[32 * j:32 * j + BL, :],
                                        xph_d[0:BL, ds(iv, 1), j * 256:(j + 1) * 256])

                psum = pspool.tile([128, 1024], mybir.dt.float32, tag="acc2")
                for H in range(NH):
                    for k in range(8):
                        for j in range(J):
                            nc.tensor.matmul(
                                psum[32 * j:32 * j + BL, 512 * H:512 * H + 384],
                                lhsT=hT_prev[:, 8 * k:8 * k + 8],
                                rhs=whs[:, (k * 4 + j) * 768 + 384 * H:(k * 4 + j) * 768 + 384 * H + 384],
                                start=(k == 0), stop=False,
                                tile_position=(0, 32 * j))
                    for j in range(J):
                        nc.tensor.matmul(
                            psum[32 * j:32 * j + BL, 512 * H:512 * H + 256],
                            lhsT=i8[:],
                            rhs=xpzr[0:8, (j * 2 + H) * 256:(j * 2 + H) * 256 + 256],
                            start=False, stop=(not with_bias),
                            tile_position=(0, 32 * j))
                    if with_bias:
                        for j in range(J):
                            nc.tensor.matmul(
                                psum[32 * j:32 * j + BL, 512 * H + 256:512 * H + 384],
                                lhsT=ones1[:],
                                rhs=brh[0:1, j * 256 + 128 * H:j * 256 + 128 * H + 128],
                                start=False, stop=True,
                                tile_position=(0, 32 * j))

                for H in range(NH):
                    o = 512 * H
                    zr_s = wpool.tile([128, 256], mybir.dt.float32, tag="zrs")
                    nc.scalar.activation(zr_s[:], psum[:, o:o + 256], AF.Sigmoid)
                    zc = wpool.tile([128, 128], mybir.dt.float32, tag="zc")
                    nc.scalar.activation(zc[:], psum[:, o:o + 128], AF.Sigmoid, scale=-1.0)
                    rhh = wpool.tile([128, 128], mybir.dt.float32, tag="rhh")
                    nc.vector.tensor_mul(rhh[:], zr_s[:, 128:256], psum[:, o + 256:o + 384])
                    rx = wpool.tile([128, 128], mybir.dt.float32, tag="rx")
                    nc.vector.tensor_add(rx[:], rhh[:], xph[:, 128 * H:128 * H + 128])
                    cand = wpool.tile([128, 128], mybir.dt.float32, tag="cand")
                    nc.scalar.activation(cand[:], rx[:], AF.Tanh)
                    # a = z*h can run while cand is being computed
                    aa = wpool.tile([128, 128], mybir.dt.float32, tag="aa")
                    nc.vector.tensor_mul(aa[:], zr_s[:, 0:128], hs_prev[:, 128 * H:128 * H + 128])
                    bb = wpool.tile([128, 128], mybir.dt.float32, tag="bb")
                    nc.vector.tensor_mul(bb[:], zc[:], cand[:])
                    nc.vector.tensor_add(hs_new[:, 128 * H:128 * H + 128], aa[:], bb[:])
                    ttmp = wpool.tile([128, 128], mybir.dt.float32, tag="ttmp")
                    nc.vector.transpose(ttmp[:], hs_new[:, 128 * H:128 * H + 128])
                    nc.vector.tensor_copy(
                        hT_new[:, 32 * H:32 * H + 32].rearrange("p (m i) -> p m i", m=4),
                        ttmp[:].rearrange("p (m i) -> p m i", m=4)[:, :, 0:8])

                # output: per-group DMA, dst u-blocks W = 4q+j
                out_r = out_d[0:BL, ds(iv, 1), :].rearrange("b t (q ji) -> b t q ji", ji=128)
                for j in range(J):
                    nc.gpsimd.dma_start(out_r[:, :, :, 32 * j:32 * j + 32],
                                        hs_new[32 * j:32 * j + BL, :])

            tc.For_i_unrolled(0, T, 1, body, max_unroll=UNROLL)

    nc.compile()
    return nc


def _get_program(with_bias):
    if with_bias not in _PROG_CACHE:
        _PROG_CACHE[with_bias] = _build_program(with_bias)
    return _PROG_CACHE[with_bias]


def kernel(x, hidden, emb, Wx, Wh, b_i, b_r):
    from concourse.bass_utils import run_bass_kernel_spmd

    x = np.asarray(x)
    hidden = np.asarray(hidden, np.float32)
    emb = np.ascontiguousarray(np.asarray(emb, np.float32))
    Wx = np.asarray(Wx, np.float32)
    Wh = np.asarray(Wh, np.float32)
    b_i = np.asarray(b_i, np.float32)
    b_r = np.asarray(b_r, np.float32)

    with_bias = bool(np.any(b_i) or np.any(b_r))
    nc = _get_program(with_bias)

    pzr, ph = _perm_zr(), _perm_h()
    wxzr_np = _round_tf32(Wx[:, pzr])
    wxh_np = _round_tf32(Wx[:, ph])
    whs_np = _make_whs(Wh)
    i8_np = np.eye(8, dtype=NP_BF16)
    ident_np = np.eye(128, dtype=np.float32)

    common = {
        "emb": emb, "wx_zr": wxzr_np, "wx_h": wxh_np, "whs": whs_np,
        "i8": i8_np, "ident": ident_np,
    }
    if with_bias:
        bias_full = b_i + b_r          # z,r gates: sigma(x + bi + h@W + br)
        common["ones_row"] = np.ones((1, 128), np.float32)
        common["bias_zr"] = _round_tf32(bias_full[pzr][None, :])
        common["bias_h"] = _round_tf32(b_i[ph][None, :])   # b_i_h only
        common["ones1"] = np.ones((1, 8), NP_BF16)
        brh_np = np.empty(1024, np.float32)
        for j in range(J):
            for H in range(NH):
                brh_np[j * 256 + 128 * H:j * 256 + 128 * H + 128] = b_r[2048 + _strip_ucols(j, H)]
        common["b_rh"] = brh_np[None, :].astype(NP_BF16)

    in_maps = []
    for c in range(NCORES):
        xs = np.ascontiguousarray(x[c * BL:(c + 1) * BL]).astype(np.int32)
        hs = hidden[c * BL:(c + 1) * BL]
        in_maps.append({
            **common,
            "x": xs,
            "maskb": (xs == 0).astype(np.float32) * MASK_BIG,
            "ht0": _ht_from_bu(hs).astype(NP_BF16),
            "hs0": _strip_from_bu(hs),
        })

    res = run_bass_kernel_spmd(nc, in_maps, list(range(NCORES)))
    global LAST_RESULTS
    LAST_RESULTS = res
    out = np.empty((B, T, U), np.float32)
    for c in range(NCORES):
        out[c * BL:(c + 1) * BL] = res.results[c]["out"]
    state = np.ascontiguousarray(out[:, -1, :])
    return out, state
